# revision 1
# baseline (speedup 1.0000x reference)
"""Trainium2 Bass kernel for nn_ContinuousDepthGeneModule (GNN message passing).

Strategy (8 NeuronCores, node-sharded):
  - Nodes partitioned contiguously across 8 cores (6250 real -> 6272 padded each).
  - All node-wise math (projections, LayerNorms, gates, RK4 glue) is local.
  - Per GCN layer: each core computes xw = cur @ W for its nodes, scales rows by
    dinv (symmetric GCN norm, folded), casts to bf16, AllGathers the full
    [50176, 128] message table, then gathers per-edge rows (dma_gather, int16
    indices into two half-tables) and segment-sums them on the TensorEngine via
    one-hot "S" matrices generated on the VectorEngine (iota + is_equal).
  - Aggregation output is node-major => LayerNorm stats/apply use per-partition
    scalars (ACT accum_out for sums, fused tensor_scalar apply).
  - One PE-transpose point per layer brings h_new back to feat-major for the
    next layer's matmuls and the gate.
  - Final global_mean_pool = per-core indicator matmul -> [G, H] partials,
    summed and normalized on host.

Self-contained: hardcodes the problem shapes; host-side prep is numpy only.
"""
import os
import sys

for _p in ("/opt/trn_rl_repo", "/root/.axon_site/_ro/trn_rl_repo"):
    if os.path.isdir(_p) and _p not in sys.path:
        sys.path.insert(0, _p)

os.environ.setdefault("MYCRO_LOCAL_CACHE", "1")

import numpy as np
import ml_dtypes
from contextlib import ExitStack

import concourse.bass as bass
import concourse.bacc as bacc
import concourse.tile as tile
from concourse import mybir
from concourse import library_config
from concourse.bass_utils import run_bass_kernel_spmd

F32 = mybir.dt.float32
BF16 = mybir.dt.bfloat16
I16 = mybir.dt.int16
AF = mybir.ActivationFunctionType
ALU = mybir.AluOpType
P = 128  # partitions


# ----------------------------------------------------------------------------
# configuration
# ----------------------------------------------------------------------------
class Cfg:
    def __init__(self, N, E, FEAT, HID, G, C, eps=1e-5, min_depth=0.1, max_depth=3.0):
        assert HID == 128, "kernel assumes hidden dim == 128"
        assert N % C == 0
        self.N, self.E, self.FEAT, self.HID, self.G, self.C = N, E, FEAT, HID, G, C
        self.eps, self.min_depth, self.max_depth = eps, min_depth, max_depth
        self.NPC = N // C                       # real nodes per core
        self.NQ = ((self.NPC + P - 1) // P) * P  # padded nodes per core
        self.B = self.NQ // P                   # dst blocks per core
        self.NTOT = self.NQ * C                 # padded total nodes
        assert self.NTOT % 2 == 0
        self.HALF = self.NTOT // 2
        assert self.HALF <= 32768, "int16 gather index limit"
        # segments of 4 blocks (512 nodes); last segment may be short
        segs = []
        b = 0
        while b < self.B:
            b1 = min(b + 4, self.B)
            segs.append((b, b1))
            b = b1
        self.segments = segs


# ----------------------------------------------------------------------------
# host-side preprocessing
# ----------------------------------------------------------------------------
def _sigmoid(x):
    return 1.0 / (1.0 + np.exp(-x))


def _pack_idx(idx):
    """int16 idx array (len % 128 == 0) -> [128, n/16] wrapped + replicated."""
    idx = np.asarray(idx, np.int16)
    n = len(idx)
    if n == 0:
        return np.zeros((128, 0), np.int16)
    arr = idx.reshape(-1, 16).T  # [16, n/16]
    return np.ascontiguousarray(np.tile(arr, (8, 1)))  # [128, n/16]


def host_prep(inputs, cfg):
    """Compute all static per-core tables + scalar constants."""
    c = cfg
    src = np.asarray(inputs["edge_index"][0], np.int64)
    dst = np.asarray(inputs["edge_index"][1], np.int64)
    batch = np.asarray(inputs["batch"], np.int64)

    # scalars
    meth_sil = float(np.mean(_sigmoid(np.asarray(inputs["methylation"], np.float64))))
    hm = _sigmoid(np.asarray(inputs["histones"], np.float64))
    act = (hm[0] + hm[2]) * 0.5
    rep = (hm[1] + hm[3]) * 0.5
    chrom = float(np.clip(act - rep + 0.5, 0.0, 1.0))
    s_scale = chrom * (1.0 - meth_sil)
    depth = float(np.clip(np.exp(float(inputs["log_depth"])), c.min_depth, c.max_depth))
    rw = float(inputs["residual_weight"])

    # gcn normalization
    deg = np.bincount(dst, minlength=c.N).astype(np.float64)
    dinv = np.where(deg > 0, 1.0 / np.sqrt(np.maximum(deg, 1.0)), 0.0).astype(np.float32)

    # edge tables
    q = dst // c.NPC
    ld = dst - q * c.NPC
    blk = ld // P
    drel = (ld % P).astype(np.float32)
    # table rows are partition-major within each core's slice: node with local
    # index l lives at row (l % 128) * B + l // 128 (matches the SBUF->DRAM
    # layout of the [128, B*H] message table slab)
    lsrc = src % c.NPC
    ps = (src // c.NPC) * c.NQ + (lsrc % P) * c.B + lsrc // P
    half = (ps >= c.HALF).astype(np.int64)
    idx16 = (ps - half * c.HALF).astype(np.int64)

    # group edges by (core, block, half)
    gid = ((q * c.B + blk) * 2 + half)
    order = np.argsort(gid, kind="stable")
    gid_s = gid[order]
    idx16_s = idx16[order]
    drel_s = drel[order]
    n_groups = c.C * c.B * 2
    counts = np.bincount(gid_s, minlength=n_groups).reshape(c.C, c.B, 2)
    starts = np.zeros(n_groups + 1, np.int64)
    np.cumsum(counts.reshape(-1), out=starts[1:])

    # uniform chunk counts per (block, half): max over cores
    KA = np.maximum.reduce(-(-counts[:, :, 0] // P), axis=0)  # [B]
    KB = np.maximum.reduce(-(-counts[:, :, 1] // P), axis=0)  # [B]
    offA = np.zeros(c.B + 1, np.int64)
    np.cumsum(KA, out=offA[1:])
    offB = np.zeros(c.B + 1, np.int64)
    np.cumsum(KB, out=offB[1:])
    totKA, totKB = int(offA[-1]), int(offB[-1])
    totK = totKA + totKB

    per_core = []
    for qq in range(c.C):
        iA = np.zeros(totKA * P, np.int64)
        iB = np.zeros(totKB * P, np.int64)
        dr = np.full((totK, P), -1.0, np.float32)
        # drel column layout must match the kernel: all A chunks (block-major)
        # at columns offA[b].., then all B chunks at totKA + offB[b]..
        for b in range(c.B):
            for h, K_b, off, iarr, dbase in ((0, int(KA[b]), int(offA[b]), iA, int(offA[b])),
                                             (1, int(KB[b]), int(offB[b]), iB,
                                              totKA + int(offB[b]))):
                g0 = starts[(qq * c.B + b) * 2 + h]
                g1 = starts[(qq * c.B + b) * 2 + h + 1]
                cnt = g1 - g0
                iarr[off * P: off * P + cnt] = idx16_s[g0:g1]
                dcols = dr[dbase: dbase + K_b].reshape(-1)
                dcols[:cnt] = drel_s[g0:g1]
        # dinv per block column
        lo, hi = qq * c.NPC, (qq + 1) * c.NPC
        dv = np.zeros(c.NQ, np.float32)
        dv[: c.NPC] = dinv[lo:hi]
        dinv_nm = np.ascontiguousarray(dv.reshape(c.B, P).T)  # [128, B]
        # pooling indicator, packed [128, B*G]
        pool = np.zeros((c.NQ, c.G), np.float32)
        bb = batch[lo:hi]
        pool[np.arange(c.NPC), bb] = 1.0
        pool_sb = np.ascontiguousarray(
            pool.reshape(c.B, P, c.G).transpose(1, 0, 2).reshape(P, c.B * c.G))
        # x feat-major + bias row
        x = np.asarray(inputs["x"], np.float32)[lo:hi]
        x_fm = np.zeros((c.FEAT + 1, c.NQ), np.float32)
        x_fm[: c.FEAT, : c.NPC] = x.T
        x_fm[c.FEAT, : c.NPC] = 1.0
        per_core.append(dict(
            x_fm=x_fm,
            idxA=_pack_idx(iA),
            idxB=_pack_idx(iB),
            drel=np.ascontiguousarray(dr.T),     # [128, totK]
            dinv_nm=dinv_nm,
            pool_ind=pool_sb,
        ))

    cnt_g = np.bincount(batch, minlength=c.G).astype(np.float32)

    meta = dict(
        KA=KA.astype(int).tolist(), KB=KB.astype(int).tolist(),
        offA=offA.astype(int).tolist(), offB=offB.astype(int).tolist(),
        totKA=totKA, totKB=totKB, totK=totK,
        s_scale=s_scale, dt=depth, rw=rw,
        cnt_g=cnt_g,
    )
    return per_core, meta


def _trivial(v, val):
    return bool(np.all(np.asarray(v) == val))


# ----------------------------------------------------------------------------
# kernel builder
# ----------------------------------------------------------------------------
def build_kernel(cfg, meta, wts):
    """wts: dict of weight arrays (values baked for zero/one specialization)."""
    c = cfg
    H = c.HID
    KA, KB = meta["KA"], meta["KB"]
    offA, offB = meta["offA"], meta["offB"]
    totKA, totKB, totK = meta["totKA"], meta["totKB"], meta["totK"]
    s_scale, dt, rw = meta["s_scale"], meta["dt"], meta["rw"]

    has_in_gb = not (_trivial(wts["in_ln_g"], 1) and _trivial(wts["in_ln_b"], 0))
    has_ln_gb = [not (_trivial(wts["ln_g"][i], 1) and _trivial(wts["ln_b"][i], 0))
                 for i in range(3)]
    has_out_gb = not (_trivial(wts["out_ln_g"], 1) and _trivial(wts["out_ln_b"], 0))
    has_gcn_b = [not _trivial(wts["gcn_b"][i], 0) for i in range(3)]
    has_out_b = not _trivial(wts["out_b"], 0)

    nc = bacc.Bacc("TRN2", target_bir_lowering=False, debug=False, num_devices=c.C)

    # external inputs
    x_in = nc.dram_tensor("x_fm", [c.FEAT + 1, c.NQ], F32, kind="ExternalInput")
    idxA_in = nc.dram_tensor("idxA", [128, totKA * 8], I16, kind="ExternalInput")
    idxB_in = nc.dram_tensor("idxB", [128, totKB * 8], I16, kind="ExternalInput")
    drel_in = nc.dram_tensor("drel", [128, totK], F32, kind="ExternalInput")
    dinv_in = nc.dram_tensor("dinv_nm", [128, c.B], F32, kind="ExternalInput")
    pool_in = nc.dram_tensor("pool_ind", [128, c.B * c.G], F32, kind="ExternalInput")
    inw_in = nc.dram_tensor("in_w65", [c.FEAT + 1, H], F32, kind="ExternalInput")
    w_in = [nc.dram_tensor(f"w{i}", [H, H], F32, kind="ExternalInput") for i in range(3)]
    gw1_in = nc.dram_tensor("gw1", [H, H], F32, kind="ExternalInput")
    gw2_in = nc.dram_tensor("gw2", [H, H], F32, kind="ExternalInput")
    outw_in = nc.dram_tensor("out_w", [H, H], F32, kind="ExternalInput")
    gateb_in = nc.dram_tensor("gate_b", [H, 1], F32, kind="ExternalInput")
    aux_rows = nc.dram_tensor("aux_rows", [1, 4 * H], F32, kind="ExternalInput")
    # aux_rows free-dim blocks: 0..2 gcn_b[i], 3 out_b
    lnp_in = nc.dram_tensor("ln_params", [128, 10 * H], F32, kind="ExternalInput")
    # ln_params columns: [in_g, in_b, l0_g, l0_b, l1_g, l1_b, l2_g, l2_b, out_g, out_b]
    pool_out = nc.dram_tensor("pool_out", [c.G, H], F32, kind="ExternalOutput")
    dbg_out = nc.dram_tensor("dbg_out", [128, c.NQ], F32, kind="ExternalOutput")

    # internal DRAM
    bounce = nc.dram_tensor("bounce", [128, c.B * H], BF16)
    table = nc.dram_tensor("table", [c.NTOT, H], BF16)
    h0_dram = nc.dram_tensor("h0_dram", [128, c.NQ], F32)
    acc_dram = nc.dram_tensor("acc_dram", [128, c.NQ], F32)

    tabA = table.ap()[0: c.HALF, :]
    tabB = table.ap()[c.HALF: c.NTOT, :]

    with tile.TileContext(nc) as tc, ExitStack() as ctx:
        const = ctx.enter_context(tc.tile_pool(name="const", bufs=1))
        big = ctx.enter_context(tc.tile_pool(name="big", bufs=1))
        st = ctx.enter_context(tc.tile_pool(name="st", bufs=3))
        stc = ctx.enter_context(tc.tile_pool(name="stc", bufs=4))
        sS = ctx.enter_context(tc.tile_pool(name="sS", bufs=4))
        gp = ctx.enter_context(tc.tile_pool(name="gp", bufs=2))
        ps_agg = ctx.enter_context(tc.tile_pool(name="ps_agg", bufs=2, space="PSUM"))
        ps_sm = ctx.enter_context(tc.tile_pool(name="ps_sm", bufs=3, space="PSUM"))

        if int(os.environ.get("GNN_LIB", "1")):
            nc.gpsimd.load_library(library_config.mlp)

        # ---- constants to SBUF
        def load_const(name, src_ap, shape, dtype=F32):
            t = const.tile(shape, dtype, tag=name)
            nc.sync.dma_start(t[:], src_ap)
            return t

        iota_t = const.tile([128, 128], F32, tag="iota")
        nc.gpsimd.iota(iota_t[:], pattern=[[1, 128]], base=0, channel_multiplier=0,
                       allow_small_or_imprecise_dtypes=True)
        pidx = const.tile([128, 1], F32, tag="pidx")
        nc.gpsimd.iota(pidx[:], pattern=[[0, 1]], base=0, channel_multiplier=1,
                       allow_small_or_imprecise_dtypes=True)
        ident = const.tile([128, 128], F32, tag="ident")
        nc.vector.tensor_scalar(ident[:], iota_t[:], pidx[:], None, ALU.is_equal)
        eps_t = const.tile([128, 1], F32, tag="eps")
        nc.vector.memset(eps_t[:], c.eps)
        ones_row = const.tile([1, 128], F32, tag="ones_row")
        nc.vector.memset(ones_row[:], 1.0)

        idxA = load_const("idxA", idxA_in[:, :], [128, totKA * 8], I16) if totKA else None
        idxB = load_const("idxB", idxB_in[:, :], [128, totKB * 8], I16) if totKB else None
        drel = load_const("drel", drel_in[:, :], [128, totK])
        dinv_nm = load_const("dinv", dinv_in[:, :], [128, c.B])
        pool_ind = load_const("pool", pool_in[:, :], [128, c.B * c.G])
        in_w65 = load_const("inw", inw_in[:, :], [c.FEAT + 1, H])
        Wt = [load_const(f"w{i}", w_in[i][:, :], [H, H]) for i in range(3)]
        gw1 = load_const("gw1", gw1_in[:, :], [H, H])
        gw2 = load_const("gw2", gw2_in[:, :], [H, H])
        out_w = load_const("outw", outw_in[:, :], [H, H])
        gate_b = load_const("gateb", gateb_in[:, :], [H, 1])
        auxr = load_const("auxr", aux_rows[:, :], [1, 4 * H])
        lnp = load_const("lnp", lnp_in[:, :], [128, 10 * H]) if (
            has_in_gb or any(has_ln_gb) or has_out_gb) else None

        # ---- persistent state
        cur_fm = big.tile([128, c.NQ], F32, tag="cur")
        y_fm = big.tile([128, c.NQ], F32, tag="y")
        xws_nm = big.tile([128, c.B * H], BF16, tag="xws")

        # ---- helpers ------------------------------------------------------
        def ln_block(src_psum, b, dinv_col, rstd_mul, gb_idx):
            """LayerNorm of one [128,128] node-major block from PSUM.

            t = src * dinv_col (per-partition, or 1.0)
            out = (t - mean) * rstd * rstd_mul (+ g/b if gb_idx)
            Returns SBUF tile [128,128] f32.
            """
            t_sb = stc.tile([128, 128], F32, tag="t")
            msum = stc.tile([128, 1], F32, tag="ms")
            if dinv_col is not None:
                nc.scalar.activation(t_sb[:], src_psum, AF.Copy,
                                     scale=dinv_col, accum_out=msum[:])
            else:
                nc.scalar.activation(t_sb[:], src_psum, AF.Copy, accum_out=msum[:])
            sq = stc.tile([128, 128], F32, tag="sq")
            ssq = stc.tile([128, 1], F32, tag="ss")
            nc.scalar.activation(sq[:], t_sb[:], AF.Square, accum_out=ssq[:])
            m = stc.tile([128, 1], F32, tag="m")
            nc.vector.tensor_scalar(m[:], msum[:], 1.0 / H, None, ALU.mult)
            m2 = stc.tile([128, 1], F32, tag="m2")
            nc.vector.tensor_tensor(m2[:], m[:], m[:], ALU.mult)
            v = stc.tile([128, 1], F32, tag="v")
            nc.vector.scalar_tensor_tensor(v[:], ssq[:], 1.0 / H, m2[:],
                                           ALU.mult, ALU.subtract)
            sd = stc.tile([128, 1], F32, tag="sd")
            nc.scalar.activation(sd[:], v[:], AF.Sqrt, bias=eps_t[:])
            rstd = stc.tile([128, 1], F32, tag="rs")
            nc.vector.reciprocal(rstd[:], sd[:])
            if rstd_mul != 1.0:
                nc.vector.tensor_scalar(rstd[:], rstd[:], float(rstd_mul), None, ALU.mult)
            hnn = st.tile([128, 128], F32, tag="hnn")
            nc.vector.tensor_scalar(hnn[:], t_sb[:], m[:], rstd[:],
                                    ALU.subtract, ALU.mult)
            if gb_idx is not None:
                g_col = lnp[:, gb_idx * 2 * H: gb_idx * 2 * H + H]
                b_col = lnp[:, gb_idx * 2 * H + H: gb_idx * 2 * H + 2 * H]
                nc.vector.tensor_tensor(hnn[:], hnn[:], g_col, ALU.mult)
                nc.vector.tensor_tensor(hnn[:], hnn[:], b_col, ALU.add)
            return hnn

        _STAGE = int(os.environ.get("GNN_STAGE", "9"))
        # ---- input projection --------------------------------------------
        x_sb = big.tile([c.FEAT + 1, c.NQ], F32, tag="x")
        nc.sync.dma_start(x_sb[:], x_in[:, :])
        for b in range(c.B):
            cols = slice(b * P, (b + 1) * P)
            ps = ps_sm.tile([128, 128], F32, tag="sm", bufs=3)
            nc.tensor.matmul(ps[:], x_sb[:, cols], in_w65[:], start=True, stop=True)
            hnn = ln_block(ps[:], b, None, 1.0, 0 if has_in_gb else None)
            tp = ps_sm.tile([128, 128], F32, tag="sm", bufs=3)
            nc.tensor.transpose(tp[:], hnn[:], ident[:])
            # relu + epigenetic scale fused into the PSUM->SBUF copy
            nc.scalar.activation(cur_fm[:, cols], tp[:], AF.Relu, scale=float(s_scale))
            nc.vector.tensor_copy(y_fm[:, cols], cur_fm[:, cols])
        nc.sync.dma_start(h0_dram[:, :], cur_fm[:])

        # ---- 12 GCN rounds ------------------------------------------------
        for r in range(12 if _STAGE >= 5 else (1 if _STAGE >= 2 else 0)):
            li, ki = r % 3, r // 3
            # stage A: xw + scaled bf16 table
            for b in range(c.B):
                cols = slice(b * P, (b + 1) * P)
                ps = ps_sm.tile([128, 128], F32, tag="sm", bufs=3)
                nc.tensor.matmul(ps[:], cur_fm[:, cols], Wt[li][:], start=True, stop=True)
                nc.vector.tensor_scalar(xws_nm[:, b * H:(b + 1) * H], ps[:],
                                        dinv_nm[:, b: b + 1], None, ALU.mult)
            nc.sync.dma_start(bounce[:, :], xws_nm[:])
            nc.gpsimd.collective_compute(
                "AllGather", ALU.bypass,
                replica_groups=[list(range(c.C))],
                ins=[bounce.ap().opt()],
                outs=[table.ap().opt()],
            )

            # stage B: gather + segment-sum + LN (+gate)
            for (b0, b1) in (c.segments if _STAGE >= 3 else []):
                nA = offA[b1] - offA[b0]
                nB = offB[b1] - offB[b0]
                width = (b1 - b0) * P
                ncols = slice(b0 * P, b0 * P + width)
                bufA = bufB = None
                GW = 2  # chunks per gather call (256 idx = proven DGE ring fit)
                if nA:
                    bufA = gp.tile([128, nA, H], BF16, tag="gA")
                    for o in range(0, nA, GW):
                        w = min(GW, nA - o)
                        c0 = offA[b0] + o
                        nc.gpsimd.dma_gather(bufA[:, o:o + w, :], tabA,
                                             idxA[:, c0 * 8: (c0 + w) * 8],
                                             w * P, w * P, H)
                if nB:
                    bufB = gp.tile([128, nB, H], BF16, tag="gB")
                    for o in range(0, nB, GW):
                        w = min(GW, nB - o)
                        c0 = offB[b0] + o
                        nc.gpsimd.dma_gather(bufB[:, o:o + w, :], tabB,
                                             idxB[:, c0 * 8: (c0 + w) * 8],
                                             w * P, w * P, H)
                if li > 0:
                    hfm_stage = st.tile([128, width], F32, tag="hfm")
                else:
                    hfm_stage = None
                for b in (range(b0, b1) if _STAGE >= 4 else []):
                    tot = KA[b] + KB[b] + (1 if has_gcn_b[li] else 0)
                    agg = ps_agg.tile([128, 128], F32, tag="agg", bufs=2)
                    k = 0
                    for src_buf, base, K_b in ((bufA, offA[b] - offA[b0], KA[b]),
                                               (bufB, offB[b] - offB[b0], KB[b])):
                        for cc in range(K_b):
                            col = (offA[b] + cc) if src_buf is bufA else (
                                totKA + offB[b] + cc)
                            S = sS.tile([128, 128], BF16, tag="S")
                            nc.vector.tensor_scalar(S[:], iota_t[:],
                                                    drel[:, col: col + 1], None,
                                                    ALU.is_equal)
                            nc.tensor.matmul(agg[:], S[:], src_buf[:, base + cc, :],
                                             start=(k == 0), stop=(k == tot - 1))
                            k += 1
                    if has_gcn_b[li]:
                        nc.tensor.matmul(agg[:], ones_row[:], auxr[:, li * H:(li + 1) * H],
                                         start=(k == 0), stop=True)
                    hnn = ln_block(agg[:], b, dinv_nm[:, b: b + 1], 1.0,
                                   (1 + li) if has_ln_gb[li] else None)
                    tp = ps_sm.tile([128, 128], F32, tag="sm", bufs=3)
                    nc.tensor.transpose(tp[:], hnn[:], ident[:])
                    if li == 0:
                        nc.scalar.activation(cur_fm[:, b * P:(b + 1) * P], tp[:], AF.Copy)
                    else:
                        nc.scalar.activation(hfm_stage[:, (b - b0) * P:(b - b0 + 1) * P],
                                             tp[:], AF.Copy)
                if li > 0:
                    gps = ps_agg.tile([128, width], F32, tag="g5", bufs=2)
                    nc.tensor.matmul(gps[:], gw1[:], cur_fm[:, ncols], start=True, stop=False)
                    nc.tensor.matmul(gps[:], gw2[:], hfm_stage[:], start=False, stop=True)
                    g_sb = st.tile([128, width], F32, tag="g")
                    nc.scalar.activation(g_sb[:], gps[:], AF.Sigmoid, bias=gate_b[:])
                    d_sb = st.tile([128, width], F32, tag="d")
                    nc.vector.tensor_tensor(d_sb[:], hfm_stage[:], cur_fm[:, ncols],
                                            ALU.subtract)
                    nc.vector.tensor_tensor(d_sb[:], g_sb[:], d_sb[:], ALU.mult)
                    nc.vector.tensor_tensor(cur_fm[:, ncols], cur_fm[:, ncols], d_sb[:],
                                            ALU.add)

            # ODE-stage boundary
            if li == 2:
                wk = [1.0, 2.0, 2.0, 1.0][ki]
                cy = [dt / 2, dt / 2, dt, 0.0][ki]
                for (b0, b1) in c.segments:
                    width = (b1 - b0) * P
                    cols = slice(b0 * P, b0 * P + width)
                    tnh = st.tile([128, width], F32, tag="bt")
                    nc.scalar.activation(tnh[:], cur_fm[:, cols], AF.Tanh)
                    kst = st.tile([128, width], F32, tag="bk")
                    nc.vector.scalar_tensor_tensor(kst[:], y_fm[:, cols], rw, tnh[:],
                                                   ALU.mult, ALU.add)
                    if ki == 0:
                        nc.sync.dma_start(acc_dram[:, cols], kst[:])
                    else:
                        ast = st.tile([128, width], F32, tag="ba")
                        nc.sync.dma_start(ast[:], acc_dram[:, cols])
                        nc.vector.scalar_tensor_tensor(ast[:], kst[:], wk, ast[:],
                                                       ALU.mult, ALU.add)
                        if ki < 3:
                            nc.sync.dma_start(acc_dram[:, cols], ast[:])
                    h0st = st.tile([128, width], F32, tag="bh")
                    nc.sync.dma_start(h0st[:], h0_dram[:, cols])
                    if ki < 3:
                        nc.vector.scalar_tensor_tensor(cur_fm[:, cols], kst[:], cy,
                                                       h0st[:], ALU.mult, ALU.add)
                        nc.vector.tensor_copy(y_fm[:, cols], cur_fm[:, cols])
                    else:
                        nc.vector.scalar_tensor_tensor(cur_fm[:, cols], ast[:], dt / 6.0,
                                                       h0st[:], ALU.mult, ALU.add)

        # ---- output projection + pooling ----------------------------------
        if _STAGE < 9:
            # bisect mode: minimal output write so outputs exist
            zst = st.tile([c.G, H], F32, tag="po")
            nc.vector.memset(zst[:], 0.0)
            nc.sync.dma_start(pool_out[:, :], zst[:])
            nc.sync.dma_start(dbg_out[:, :], cur_fm[:])
            return nc
        pool_ps = ps_agg.tile([c.G, H], F32, tag="pool", bufs=1)
        for b in range(c.B):
            cols = slice(b * P, (b + 1) * P)
            ps = ps_sm.tile([128, 128], F32, tag="sm", bufs=3)
            nc.tensor.matmul(ps[:], cur_fm[:, cols], out_w[:], start=True,
                             stop=not has_out_b)
            if has_out_b:
                nc.tensor.matmul(ps[:], ones_row[:], auxr[:, 3 * H: 4 * H], start=False, stop=True)
            hnn = ln_block(ps[:], b, None, 1.0, 4 if has_out_gb else None)
            nc.tensor.matmul(pool_ps[:], pool_ind[:, b * c.G:(b + 1) * c.G], hnn[:],
                             start=(b == 0), stop=(b == c.B - 1))
        pool_sb = st.tile([c.G, H], F32, tag="po")
        nc.vector.tensor_copy(pool_sb[:], pool_ps[:])
        nc.sync.dma_start(pool_out[:, :], pool_sb[:])
        nc.sync.dma_start(dbg_out[:, :], cur_fm[:])

    return nc


# ----------------------------------------------------------------------------
# entry point
# ----------------------------------------------------------------------------
_CACHE = {}
LAST_EXEC_NS = None
LAST_RESULTS = None


def _weights_pack(inputs, cfg):
    c = cfg
    in_w = np.asarray(inputs["in_w"], np.float32)
    in_b = np.asarray(inputs["in_b"], np.float32)
    in_w65 = np.concatenate([in_w, in_b[None, :]], axis=0)
    gate_w = np.asarray(inputs["gate_w"], np.float32)
    aux = np.zeros((1, 4 * c.HID), np.float32)
    aux[0, : 3 * c.HID] = np.asarray(inputs["gcn_b"], np.float32).reshape(-1)
    aux[0, 3 * c.HID:] = np.asarray(inputs["out_b"], np.float32)
    lnp = np.zeros((128, 10 * c.HID), np.float32)
    seq = [inputs["in_ln_g"], inputs["in_ln_b"],
           inputs["ln_g"][0], inputs["ln_b"][0],
           inputs["ln_g"][1], inputs["ln_b"][1],
           inputs["ln_g"][2], inputs["ln_b"][2],
           inputs["out_ln_g"], inputs["out_ln_b"]]
    for i, v in enumerate(seq):
        lnp[:, i * c.HID:(i + 1) * c.HID] = np.asarray(v, np.float32)[None, :]
    return dict(
        in_w65=in_w65,
        w=[np.ascontiguousarray(np.asarray(inputs["gcn_w"], np.float32)[i])
           for i in range(3)],
        gw1=np.ascontiguousarray(gate_w[: c.HID]),
        gw2=np.ascontiguousarray(gate_w[c.HID:]),
        out_w=np.asarray(inputs["out_w"], np.float32),
        gate_b=np.asarray(inputs["gate_b"], np.float32).reshape(c.HID, 1),
        aux_rows=aux,
        ln_params=lnp,
        # raw (for specialization flags)
        in_ln_g=inputs["in_ln_g"], in_ln_b=inputs["in_ln_b"],
        ln_g=np.asarray(inputs["ln_g"]), ln_b=np.asarray(inputs["ln_b"]),
        out_ln_g=inputs["out_ln_g"], out_ln_b=inputs["out_ln_b"],
        gcn_b=np.asarray(inputs["gcn_b"]), out_b=inputs["out_b"],
    )


def kernel_impl(inputs, cfg, profile=False):
    global LAST_EXEC_NS, LAST_RESULTS
    inputs = {k: np.asarray(v) for k, v in inputs.items()}
    per_core, meta = host_prep(inputs, cfg)
    wts = _weights_pack(inputs, cfg)

    key = (cfg.N, cfg.E, cfg.C,
           hash(inputs["edge_index"].tobytes()),
           hash(inputs["batch"].tobytes()),
           meta["s_scale"], meta["dt"], meta["rw"])
    if key not in _CACHE:
        nc = build_kernel(cfg, meta, wts)
        if not nc.is_finalized():
            nc.finalize()
        _CACHE.clear()
        _CACHE[key] = nc
    nc = _CACHE[key]

    in_maps = []
    for q in range(cfg.C):
        m = dict(per_core[q])
        m["in_w65"] = wts["in_w65"]
        for i in range(3):
            m[f"w{i}"] = wts["w"][i]
        m["gw1"] = wts["gw1"]
        m["gw2"] = wts["gw2"]
        m["out_w"] = wts["out_w"]
        m["gate_b"] = wts["gate_b"]
        m["aux_rows"] = wts["aux_rows"]
        m["ln_params"] = wts["ln_params"]
        m["idxA"] = m.pop("idxA")
        m["idxB"] = m.pop("idxB")
        m = {k: (v.astype(ml_dtypes.bfloat16) if k in () else v) for k, v in m.items()}
        in_maps.append(m)

    res = run_bass_kernel_spmd(nc, in_maps, core_ids=list(range(cfg.C)),
                               trace=profile)
    LAST_RESULTS = res
    LAST_EXEC_NS = res.exec_time_ns

    pooled = np.zeros((cfg.G, cfg.HID), np.float64)
    for q in range(cfg.C):
        pooled += np.asarray(res.results[q]["pool_out"], np.float64)
    cnt = np.maximum(meta["cnt_g"], 1.0)
    out = (pooled / cnt[:, None]).astype(np.float32)
    return out


def kernel(**inputs):
    cfg = Cfg(N=50000, E=800000, FEAT=64, HID=128, G=8, C=8)
    profile = bool(int(os.environ.get("GNN_PROFILE", "0")))
    return kernel_impl(inputs, cfg, profile=profile)



# revision 11
# speedup vs baseline: 1.2211x; 1.2211x over previous
"""Trainium2 Bass kernel for nn_ContinuousDepthGeneModule (GNN message passing).

Strategy (8 NeuronCores, node-sharded):
  - Nodes partitioned contiguously across 8 cores (6250 real -> 6272 padded each).
  - All node-wise math (projections, LayerNorms, gates, RK4 glue) is local.
  - Per GCN layer: each core computes xw = cur @ W for its nodes, scales rows by
    dinv (symmetric GCN norm, folded), casts to bf16, AllGathers the full
    [50176, 128] message table, then gathers per-edge rows (dma_gather, int16
    indices into two half-tables) and segment-sums them on the TensorEngine via
    one-hot "S" matrices generated on the VectorEngine (iota + is_equal).
  - Aggregation output is node-major => LayerNorm stats/apply use per-partition
    scalars (ACT accum_out for sums, fused tensor_scalar apply).
  - One PE-transpose point per layer brings h_new back to feat-major for the
    next layer's matmuls and the gate.
  - Final global_mean_pool = per-core indicator matmul -> [G, H] partials,
    summed and normalized on host.

Self-contained: hardcodes the problem shapes; host-side prep is numpy only.
"""
import os
import sys

for _p in ("/opt/trn_rl_repo", "/root/.axon_site/_ro/trn_rl_repo"):
    if os.path.isdir(_p) and _p not in sys.path:
        sys.path.insert(0, _p)

os.environ.setdefault("MYCRO_LOCAL_CACHE", "1")

import numpy as np
import ml_dtypes
from contextlib import ExitStack

import concourse.bass as bass
import concourse.bacc as bacc
import concourse.tile as tile
from concourse import mybir
from concourse import library_config
from concourse.bass_utils import run_bass_kernel_spmd

F32 = mybir.dt.float32
BF16 = mybir.dt.bfloat16
I16 = mybir.dt.int16
AF = mybir.ActivationFunctionType
ALU = mybir.AluOpType
P = 128  # partitions


# ----------------------------------------------------------------------------
# configuration
# ----------------------------------------------------------------------------
class Cfg:
    def __init__(self, N, E, FEAT, HID, G, C, eps=1e-5, min_depth=0.1, max_depth=3.0):
        assert HID == 128, "kernel assumes hidden dim == 128"
        assert N % C == 0
        self.N, self.E, self.FEAT, self.HID, self.G, self.C = N, E, FEAT, HID, G, C
        self.eps, self.min_depth, self.max_depth = eps, min_depth, max_depth
        self.NPC = N // C                       # real nodes per core
        self.NQ = ((self.NPC + P - 1) // P) * P  # padded nodes per core
        self.B = self.NQ // P                   # dst blocks per core
        self.NTOT = self.NQ * C                 # padded total nodes
        assert self.NTOT % 2 == 0
        self.HALF = self.NTOT // 2
        assert self.HALF <= 32768, "int16 gather index limit"
        # segments of 4 blocks (512 nodes); last segment may be short
        segs = []
        b = 0
        while b < self.B:
            b1 = min(b + 4, self.B)
            segs.append((b, b1))
            b = b1
        self.segments = segs


# ----------------------------------------------------------------------------
# host-side preprocessing
# ----------------------------------------------------------------------------
def _sigmoid(x):
    return 1.0 / (1.0 + np.exp(-x))


def _pack_idx(idx):
    """int16 idx array (len % 128 == 0) -> [128, n/16] wrapped + replicated."""
    idx = np.asarray(idx, np.int16)
    n = len(idx)
    if n == 0:
        return np.zeros((128, 0), np.int16)
    arr = idx.reshape(-1, 16).T  # [16, n/16]
    return np.ascontiguousarray(np.tile(arr, (8, 1)))  # [128, n/16]


def host_prep(inputs, cfg):
    """Compute all static per-core tables + scalar constants."""
    c = cfg
    src = np.asarray(inputs["edge_index"][0], np.int64)
    dst = np.asarray(inputs["edge_index"][1], np.int64)
    batch = np.asarray(inputs["batch"], np.int64)

    # scalars
    meth_sil = float(np.mean(_sigmoid(np.asarray(inputs["methylation"], np.float64))))
    hm = _sigmoid(np.asarray(inputs["histones"], np.float64))
    act = (hm[0] + hm[2]) * 0.5
    rep = (hm[1] + hm[3]) * 0.5
    chrom = float(np.clip(act - rep + 0.5, 0.0, 1.0))
    s_scale = chrom * (1.0 - meth_sil)
    depth = float(np.clip(np.exp(float(inputs["log_depth"])), c.min_depth, c.max_depth))
    rw = float(inputs["residual_weight"])

    # gcn normalization
    deg = np.bincount(dst, minlength=c.N).astype(np.float64)
    dinv = np.where(deg > 0, 1.0 / np.sqrt(np.maximum(deg, 1.0)), 0.0).astype(np.float32)

    # edge tables
    q = dst // c.NPC
    ld = dst - q * c.NPC
    blk = ld // P
    drel = (ld % P).astype(np.float32)
    # table rows are partition-major within each core's slice: node with local
    # index l lives at row (l % 128) * B + l // 128 (matches the SBUF->DRAM
    # layout of the [128, B*H] message table slab)
    lsrc = src % c.NPC
    ps = (src // c.NPC) * c.NQ + (lsrc % P) * c.B + lsrc // P
    half = (ps >= c.HALF).astype(np.int64)
    idx16 = (ps - half * c.HALF).astype(np.int64)

    # group edges by (core, block, half)
    gid = ((q * c.B + blk) * 2 + half)
    order = np.argsort(gid, kind="stable")
    gid_s = gid[order]
    idx16_s = idx16[order]
    drel_s = drel[order]
    n_groups = c.C * c.B * 2
    counts = np.bincount(gid_s, minlength=n_groups).reshape(c.C, c.B, 2)
    starts = np.zeros(n_groups + 1, np.int64)
    np.cumsum(counts.reshape(-1), out=starts[1:])

    # uniform chunk counts per (block, half): max over cores
    KA = np.maximum.reduce(-(-counts[:, :, 0] // P), axis=0)  # [B]
    KB = np.maximum.reduce(-(-counts[:, :, 1] // P), axis=0)  # [B]
    offA = np.zeros(c.B + 1, np.int64)
    np.cumsum(KA, out=offA[1:])
    offB = np.zeros(c.B + 1, np.int64)
    np.cumsum(KB, out=offB[1:])
    totKA, totKB = int(offA[-1]), int(offB[-1])
    totK = totKA + totKB

    per_core = []
    for qq in range(c.C):
        iA = np.zeros(totKA * P, np.int64)
        iB = np.zeros(totKB * P, np.int64)
        dr = np.full((totK, P), -1.0, np.float32)
        # drel column layout must match the kernel: all A chunks (block-major)
        # at columns offA[b].., then all B chunks at totKA + offB[b]..
        for b in range(c.B):
            for h, K_b, off, iarr, dbase in ((0, int(KA[b]), int(offA[b]), iA, int(offA[b])),
                                             (1, int(KB[b]), int(offB[b]), iB,
                                              totKA + int(offB[b]))):
                g0 = starts[(qq * c.B + b) * 2 + h]
                g1 = starts[(qq * c.B + b) * 2 + h + 1]
                cnt = g1 - g0
                iarr[off * P: off * P + cnt] = idx16_s[g0:g1]
                dcols = dr[dbase: dbase + K_b].reshape(-1)
                dcols[:cnt] = drel_s[g0:g1]
        # dinv per block column
        lo, hi = qq * c.NPC, (qq + 1) * c.NPC
        dv = np.zeros(c.NQ, np.float32)
        dv[: c.NPC] = dinv[lo:hi]
        dinv_nm = np.ascontiguousarray(dv.reshape(c.B, P).T)  # [128, B]
        # pooling indicator, packed [128, B*G]
        pool = np.zeros((c.NQ, c.G), np.float32)
        bb = batch[lo:hi]
        pool[np.arange(c.NPC), bb] = 1.0
        pool_sb = np.ascontiguousarray(
            pool.reshape(c.B, P, c.G).transpose(1, 0, 2).reshape(P, c.B * c.G))
        # x feat-major + bias row
        x = np.asarray(inputs["x"], np.float32)[lo:hi]
        x_fm = np.zeros((c.FEAT + 1, c.NQ), np.float32)
        x_fm[: c.FEAT, : c.NPC] = x.T
        x_fm[c.FEAT, : c.NPC] = 1.0
        per_core.append(dict(
            x_fm=x_fm,
            idxA=_pack_idx(iA),
            idxB=_pack_idx(iB),
            drel=np.ascontiguousarray(dr.T),     # [128, totK]
            dinv_nm=dinv_nm,
            pool_ind=pool_sb,
        ))

    cnt_g = np.bincount(batch, minlength=c.G).astype(np.float32)

    meta = dict(
        KA=KA.astype(int).tolist(), KB=KB.astype(int).tolist(),
        offA=offA.astype(int).tolist(), offB=offB.astype(int).tolist(),
        totKA=totKA, totKB=totKB, totK=totK,
        s_scale=s_scale, dt=depth, rw=rw,
        cnt_g=cnt_g,
    )
    return per_core, meta


def _trivial(v, val):
    return bool(np.all(np.asarray(v) == val))


# ----------------------------------------------------------------------------
# kernel builder
# ----------------------------------------------------------------------------
def build_kernel(cfg, meta, wts):
    """wts: dict of weight arrays (values baked for zero/one specialization)."""
    c = cfg
    H = c.HID
    KA, KB = meta["KA"], meta["KB"]
    offA, offB = meta["offA"], meta["offB"]
    totKA, totKB, totK = meta["totKA"], meta["totKB"], meta["totK"]
    s_scale, dt, rw = meta["s_scale"], meta["dt"], meta["rw"]

    has_in_gb = not (_trivial(wts["in_ln_g"], 1) and _trivial(wts["in_ln_b"], 0))
    has_ln_gb = [not (_trivial(wts["ln_g"][i], 1) and _trivial(wts["ln_b"][i], 0))
                 for i in range(3)]
    has_out_gb = not (_trivial(wts["out_ln_g"], 1) and _trivial(wts["out_ln_b"], 0))
    has_gcn_b = [not _trivial(wts["gcn_b"][i], 0) for i in range(3)]
    has_out_b = not _trivial(wts["out_b"], 0)

    nc = bacc.Bacc("TRN2", target_bir_lowering=False, debug=False, num_devices=c.C)

    # external inputs
    x_in = nc.dram_tensor("x_fm", [c.FEAT + 1, c.NQ], F32, kind="ExternalInput")
    idxA_in = nc.dram_tensor("idxA", [128, totKA * 8], I16, kind="ExternalInput")
    idxB_in = nc.dram_tensor("idxB", [128, totKB * 8], I16, kind="ExternalInput")
    drel_in = nc.dram_tensor("drel", [128, totK], F32, kind="ExternalInput")
    dinv_in = nc.dram_tensor("dinv_nm", [128, c.B], F32, kind="ExternalInput")
    pool_in = nc.dram_tensor("pool_ind", [128, c.B * c.G], F32, kind="ExternalInput")
    inw_in = nc.dram_tensor("in_w65", [c.FEAT + 1, H], F32, kind="ExternalInput")
    w_in = [nc.dram_tensor(f"w{i}", [H, H], F32, kind="ExternalInput") for i in range(3)]
    gw1_in = nc.dram_tensor("gw1", [H, H], F32, kind="ExternalInput")
    gw2_in = nc.dram_tensor("gw2", [H, H], F32, kind="ExternalInput")
    outw_in = nc.dram_tensor("out_w", [H, H], F32, kind="ExternalInput")
    gateb_in = nc.dram_tensor("gate_b", [H, 1], F32, kind="ExternalInput")
    aux_rows = nc.dram_tensor("aux_rows", [1, 4 * H], F32, kind="ExternalInput")
    # aux_rows free-dim blocks: 0..2 gcn_b[i], 3 out_b
    lnp_in = nc.dram_tensor("ln_params", [128, 10 * H], F32, kind="ExternalInput")
    # ln_params columns: [in_g, in_b, l0_g, l0_b, l1_g, l1_b, l2_g, l2_b, out_g, out_b]
    pool_out = nc.dram_tensor("pool_out", [c.G, H], F32, kind="ExternalOutput")
    dbg_out = nc.dram_tensor("dbg_out", [128, c.NQ], F32, kind="ExternalOutput")

    # internal DRAM
    bounce = nc.dram_tensor("bounce", [128, c.B * H], BF16)
    table = nc.dram_tensor("table", [c.NTOT, H], BF16, addr_space="Shared")
    h0_dram = nc.dram_tensor("h0_dram", [128, c.NQ], F32)
    acc_dram = nc.dram_tensor("acc_dram", [128, c.NQ], F32)

    tabA = table.ap()[0: c.HALF, :]
    tabB = table.ap()[c.HALF: c.NTOT, :]

    with tile.TileContext(nc) as tc, ExitStack() as ctx:
        const = ctx.enter_context(tc.tile_pool(name="const", bufs=1))
        big = ctx.enter_context(tc.tile_pool(name="big", bufs=1))
        st = ctx.enter_context(tc.tile_pool(name="st", bufs=3))
        stc = ctx.enter_context(tc.tile_pool(name="stc", bufs=4))
        sS = ctx.enter_context(tc.tile_pool(name="sS", bufs=4))
        gp = ctx.enter_context(tc.tile_pool(name="gp", bufs=2))
        ps_agg = ctx.enter_context(tc.tile_pool(name="ps_agg", bufs=2, space="PSUM"))
        ps_sm = ctx.enter_context(tc.tile_pool(name="ps_sm", bufs=3, space="PSUM"))

        if int(os.environ.get("GNN_LIB", "1")):
            nc.gpsimd.load_library(library_config.mlp)

        # ---- constants to SBUF
        def load_const(name, src_ap, shape, dtype=F32):
            t = const.tile(shape, dtype, tag=name)
            nc.sync.dma_start(t[:], src_ap)
            return t

        iota_t = const.tile([128, 128], F32, tag="iota")
        nc.gpsimd.iota(iota_t[:], pattern=[[1, 128]], base=0, channel_multiplier=0,
                       allow_small_or_imprecise_dtypes=True)
        # bf16 iota for one-hot builds: 16-bit in/out doubles DVE throughput
        iota_bf = const.tile([128, 128], BF16, tag="iota_bf")
        nc.gpsimd.iota(iota_bf[:], pattern=[[1, 128]], base=0, channel_multiplier=0,
                       allow_small_or_imprecise_dtypes=True)
        pidx = const.tile([128, 1], F32, tag="pidx")
        nc.gpsimd.iota(pidx[:], pattern=[[0, 1]], base=0, channel_multiplier=1,
                       allow_small_or_imprecise_dtypes=True)
        ident = const.tile([128, 128], F32, tag="ident")
        nc.vector.tensor_scalar(ident[:], iota_t[:], pidx[:], None, ALU.is_equal)
        eps_t = const.tile([128, 1], F32, tag="eps")
        nc.vector.memset(eps_t[:], c.eps)
        ones_row = const.tile([1, 128], F32, tag="ones_row")
        nc.vector.memset(ones_row[:], 1.0)

        idxA = load_const("idxA", idxA_in[:, :], [128, totKA * 8], I16) if totKA else None
        idxB = load_const("idxB", idxB_in[:, :], [128, totKB * 8], I16) if totKB else None
        drel = load_const("drel", drel_in[:, :], [128, totK])
        dinv_nm = load_const("dinv", dinv_in[:, :], [128, c.B])
        pool_ind = load_const("pool", pool_in[:, :], [128, c.B * c.G])
        in_w65 = load_const("inw", inw_in[:, :], [c.FEAT + 1, H])
        Wt = [load_const(f"w{i}", w_in[i][:, :], [H, H]) for i in range(3)]
        gw1 = load_const("gw1", gw1_in[:, :], [H, H])
        gw2 = load_const("gw2", gw2_in[:, :], [H, H])
        out_w = load_const("outw", outw_in[:, :], [H, H])
        gate_b = load_const("gateb", gateb_in[:, :], [H, 1])
        auxr = load_const("auxr", aux_rows[:, :], [1, 4 * H])
        lnp = load_const("lnp", lnp_in[:, :], [128, 10 * H]) if (
            has_in_gb or any(has_ln_gb) or has_out_gb) else None

        # ---- persistent state
        cur_fm = big.tile([128, c.NQ], F32, tag="cur")
        y_fm = big.tile([128, c.NQ], F32, tag="y")
        xws_nm = big.tile([128, c.B * H], BF16, tag="xws")

        # ---- helpers ------------------------------------------------------
        def ln_block(src_psum, b, dinv_col, rstd_mul, gb_idx):
            """LayerNorm of one [128,128] node-major block from PSUM.

            t = src * dinv_col (per-partition, or 1.0)
            out = (t - mean) * rstd * rstd_mul (+ g/b if gb_idx)
            Returns SBUF tile [128,128] f32.
            """
            t_sb = stc.tile([128, 128], F32, tag="t")
            msum = stc.tile([128, 1], F32, tag="ms")
            if dinv_col is not None:
                nc.scalar.activation(t_sb[:], src_psum, AF.Copy,
                                     scale=dinv_col, accum_out=msum[:])
            else:
                nc.scalar.activation(t_sb[:], src_psum, AF.Copy, accum_out=msum[:])
            sq = stc.tile([128, 128], F32, tag="sq")
            ssq = stc.tile([128, 1], F32, tag="ss")
            nc.scalar.activation(sq[:], t_sb[:], AF.Square, accum_out=ssq[:])
            m = stc.tile([128, 1], F32, tag="m")
            nc.vector.tensor_scalar(m[:], msum[:], 1.0 / H, None, ALU.mult)
            m2 = stc.tile([128, 1], F32, tag="m2")
            nc.vector.tensor_tensor(m2[:], m[:], m[:], ALU.mult)
            v = stc.tile([128, 1], F32, tag="v")
            nc.vector.scalar_tensor_tensor(v[:], ssq[:], 1.0 / H, m2[:],
                                           ALU.mult, ALU.subtract)
            sd = stc.tile([128, 1], F32, tag="sd")
            nc.scalar.activation(sd[:], v[:], AF.Sqrt, bias=eps_t[:])
            rstd = stc.tile([128, 1], F32, tag="rs")
            nc.vector.reciprocal(rstd[:], sd[:])
            if rstd_mul != 1.0:
                nc.vector.tensor_scalar(rstd[:], rstd[:], float(rstd_mul), None, ALU.mult)
            hnn = st.tile([128, 128], F32, tag="hnn")
            nc.vector.tensor_scalar(hnn[:], t_sb[:], m[:], rstd[:],
                                    ALU.subtract, ALU.mult)
            if gb_idx is not None:
                g_col = lnp[:, gb_idx * 2 * H: gb_idx * 2 * H + H]
                b_col = lnp[:, gb_idx * 2 * H + H: gb_idx * 2 * H + 2 * H]
                nc.vector.tensor_tensor(hnn[:], hnn[:], g_col, ALU.mult)
                nc.vector.tensor_tensor(hnn[:], hnn[:], b_col, ALU.add)
            return hnn

        _STAGE = int(os.environ.get("GNN_STAGE", "9"))
        # ---- input projection --------------------------------------------
        x_sb = big.tile([c.FEAT + 1, c.NQ], F32, tag="x")
        nc.sync.dma_start(x_sb[:], x_in[:, :])
        for b in range(c.B):
            cols = slice(b * P, (b + 1) * P)
            ps = ps_sm.tile([128, 128], F32, tag="sm", bufs=3)
            nc.tensor.matmul(ps[:], x_sb[:, cols], in_w65[:], start=True, stop=True)
            hnn = ln_block(ps[:], b, None, 1.0, 0 if has_in_gb else None)
            tp = ps_sm.tile([128, 128], F32, tag="sm", bufs=3)
            nc.tensor.transpose(tp[:], hnn[:], ident[:])
            # relu + epigenetic scale fused into the PSUM->SBUF copy
            nc.scalar.activation(cur_fm[:, cols], tp[:], AF.Relu, scale=float(s_scale))
            nc.vector.tensor_copy(y_fm[:, cols], cur_fm[:, cols])
        nc.sync.dma_start(h0_dram[:, :], cur_fm[:])

        # ---- 12 GCN rounds ------------------------------------------------
        for r in range(12 if _STAGE >= 5 else (1 if _STAGE >= 2 else 0)):
            li, ki = r % 3, r // 3
            # stage A: xw + scaled bf16 table
            for b in range(c.B):
                cols = slice(b * P, (b + 1) * P)
                ps = ps_sm.tile([128, 128], F32, tag="sm", bufs=3)
                nc.tensor.matmul(ps[:], cur_fm[:, cols], Wt[li][:], start=True, stop=True)
                nc.vector.tensor_scalar(xws_nm[:, b * H:(b + 1) * H], ps[:],
                                        dinv_nm[:, b: b + 1], None, ALU.mult)
            nc.sync.dma_start(bounce[:, :], xws_nm[:])
            nc.gpsimd.collective_compute(
                "AllGather", ALU.bypass,
                replica_groups=[list(range(c.C))],
                ins=[bounce.ap().opt()],
                outs=[table.ap().opt()],
            )

            # stage B: gather + segment-sum + LN (+gate)
            for (b0, b1) in (c.segments if _STAGE >= 3 else []):
                nA = offA[b1] - offA[b0]
                nB = offB[b1] - offB[b0]
                width = (b1 - b0) * P
                ncols = slice(b0 * P, b0 * P + width)
                bufA = bufB = None
                GW = int(os.environ.get("GNN_GW", "8"))  # chunks per gather call
                if nA:
                    bufA = gp.tile([128, nA, H], BF16, tag="gA")
                    for o in range(0, nA, GW):
                        w = min(GW, nA - o)
                        c0 = offA[b0] + o
                        nc.gpsimd.dma_gather(bufA[:, o:o + w, :], tabA,
                                             idxA[:, c0 * 8: (c0 + w) * 8],
                                             w * P, w * P, H)
                if nB:
                    bufB = gp.tile([128, nB, H], BF16, tag="gB")
                    for o in range(0, nB, GW):
                        w = min(GW, nB - o)
                        c0 = offB[b0] + o
                        nc.gpsimd.dma_gather(bufB[:, o:o + w, :], tabB,
                                             idxB[:, c0 * 8: (c0 + w) * 8],
                                             w * P, w * P, H)
                if li > 0:
                    hfm_stage = st.tile([128, width], F32, tag="hfm")
                else:
                    hfm_stage = None
                for b in (range(b0, b1) if _STAGE >= 4 else []):
                    tot = KA[b] + KB[b] + (1 if has_gcn_b[li] else 0)
                    agg = ps_agg.tile([128, 128], F32, tag="agg", bufs=2)
                    k = 0
                    for src_buf, base, K_b in ((bufA, offA[b] - offA[b0], KA[b]),
                                               (bufB, offB[b] - offB[b0], KB[b])):
                        for cc in range(K_b):
                            col = (offA[b] + cc) if src_buf is bufA else (
                                totKA + offB[b] + cc)
                            S = sS.tile([128, 128], BF16, tag="S")
                            nc.vector.tensor_scalar(S[:], iota_bf[:],
                                                    drel[:, col: col + 1], None,
                                                    ALU.is_equal)
                            nc.tensor.matmul(agg[:], S[:], src_buf[:, base + cc, :],
                                             start=(k == 0), stop=(k == tot - 1))
                            k += 1
                    if has_gcn_b[li]:
                        nc.tensor.matmul(agg[:], ones_row[:], auxr[:, li * H:(li + 1) * H],
                                         start=(k == 0), stop=True)
                    hnn = ln_block(agg[:], b, dinv_nm[:, b: b + 1], 1.0,
                                   (1 + li) if has_ln_gb[li] else None)
                    tp = ps_sm.tile([128, 128], F32, tag="sm", bufs=3)
                    nc.tensor.transpose(tp[:], hnn[:], ident[:])
                    if li == 0:
                        nc.scalar.activation(cur_fm[:, b * P:(b + 1) * P], tp[:], AF.Copy)
                    else:
                        nc.scalar.activation(hfm_stage[:, (b - b0) * P:(b - b0 + 1) * P],
                                             tp[:], AF.Copy)
                if li > 0:
                    gps = ps_agg.tile([128, width], F32, tag="g5", bufs=2)
                    nc.tensor.matmul(gps[:], gw1[:], cur_fm[:, ncols], start=True, stop=False)
                    nc.tensor.matmul(gps[:], gw2[:], hfm_stage[:], start=False, stop=True)
                    g_sb = st.tile([128, width], F32, tag="g")
                    nc.scalar.activation(g_sb[:], gps[:], AF.Sigmoid, bias=gate_b[:])
                    d_sb = st.tile([128, width], F32, tag="d")
                    nc.vector.tensor_tensor(d_sb[:], hfm_stage[:], cur_fm[:, ncols],
                                            ALU.subtract)
                    nc.vector.tensor_tensor(d_sb[:], g_sb[:], d_sb[:], ALU.mult)
                    nc.vector.tensor_tensor(cur_fm[:, ncols], cur_fm[:, ncols], d_sb[:],
                                            ALU.add)

            # ODE-stage boundary
            if li == 2:
                wk = [1.0, 2.0, 2.0, 1.0][ki]
                cy = [dt / 2, dt / 2, dt, 0.0][ki]
                for (b0, b1) in c.segments:
                    width = (b1 - b0) * P
                    cols = slice(b0 * P, b0 * P + width)
                    tnh = st.tile([128, width], F32, tag="bt")
                    nc.scalar.activation(tnh[:], cur_fm[:, cols], AF.Tanh)
                    kst = st.tile([128, width], F32, tag="bk")
                    nc.vector.scalar_tensor_tensor(kst[:], y_fm[:, cols], rw, tnh[:],
                                                   ALU.mult, ALU.add)
                    if ki == 0:
                        nc.sync.dma_start(acc_dram[:, cols], kst[:])
                    else:
                        ast = st.tile([128, width], F32, tag="ba")
                        nc.sync.dma_start(ast[:], acc_dram[:, cols])
                        nc.vector.scalar_tensor_tensor(ast[:], kst[:], wk, ast[:],
                                                       ALU.mult, ALU.add)
                        if ki < 3:
                            nc.sync.dma_start(acc_dram[:, cols], ast[:])
                    h0st = st.tile([128, width], F32, tag="bh")
                    nc.sync.dma_start(h0st[:], h0_dram[:, cols])
                    if ki < 3:
                        nc.vector.scalar_tensor_tensor(cur_fm[:, cols], kst[:], cy,
                                                       h0st[:], ALU.mult, ALU.add)
                        nc.vector.tensor_copy(y_fm[:, cols], cur_fm[:, cols])
                    else:
                        nc.vector.scalar_tensor_tensor(cur_fm[:, cols], ast[:], dt / 6.0,
                                                       h0st[:], ALU.mult, ALU.add)

        # ---- output projection + pooling ----------------------------------
        if _STAGE < 9:
            # bisect mode: minimal output write so outputs exist
            zst = st.tile([c.G, H], F32, tag="po")
            nc.vector.memset(zst[:], 0.0)
            nc.sync.dma_start(pool_out[:, :], zst[:])
            nc.sync.dma_start(dbg_out[:, :], cur_fm[:])
            return nc
        pool_ps = ps_agg.tile([c.G, H], F32, tag="pool", bufs=1)
        for b in range(c.B):
            cols = slice(b * P, (b + 1) * P)
            ps = ps_sm.tile([128, 128], F32, tag="sm", bufs=3)
            nc.tensor.matmul(ps[:], cur_fm[:, cols], out_w[:], start=True,
                             stop=not has_out_b)
            if has_out_b:
                nc.tensor.matmul(ps[:], ones_row[:], auxr[:, 3 * H: 4 * H], start=False, stop=True)
            hnn = ln_block(ps[:], b, None, 1.0, 4 if has_out_gb else None)
            nc.tensor.matmul(pool_ps[:], pool_ind[:, b * c.G:(b + 1) * c.G], hnn[:],
                             start=(b == 0), stop=(b == c.B - 1))
        pool_sb = st.tile([c.G, H], F32, tag="po")
        nc.vector.tensor_copy(pool_sb[:], pool_ps[:])
        nc.sync.dma_start(pool_out[:, :], pool_sb[:])
        nc.sync.dma_start(dbg_out[:, :], cur_fm[:])

    return nc


# ----------------------------------------------------------------------------
# entry point
# ----------------------------------------------------------------------------
_CACHE = {}
LAST_EXEC_NS = None
LAST_RESULTS = None


def _weights_pack(inputs, cfg):
    c = cfg
    in_w = np.asarray(inputs["in_w"], np.float32)
    in_b = np.asarray(inputs["in_b"], np.float32)
    in_w65 = np.concatenate([in_w, in_b[None, :]], axis=0)
    gate_w = np.asarray(inputs["gate_w"], np.float32)
    aux = np.zeros((1, 4 * c.HID), np.float32)
    aux[0, : 3 * c.HID] = np.asarray(inputs["gcn_b"], np.float32).reshape(-1)
    aux[0, 3 * c.HID:] = np.asarray(inputs["out_b"], np.float32)
    lnp = np.zeros((128, 10 * c.HID), np.float32)
    seq = [inputs["in_ln_g"], inputs["in_ln_b"],
           inputs["ln_g"][0], inputs["ln_b"][0],
           inputs["ln_g"][1], inputs["ln_b"][1],
           inputs["ln_g"][2], inputs["ln_b"][2],
           inputs["out_ln_g"], inputs["out_ln_b"]]
    for i, v in enumerate(seq):
        lnp[:, i * c.HID:(i + 1) * c.HID] = np.asarray(v, np.float32)[None, :]
    return dict(
        in_w65=in_w65,
        w=[np.ascontiguousarray(np.asarray(inputs["gcn_w"], np.float32)[i])
           for i in range(3)],
        gw1=np.ascontiguousarray(gate_w[: c.HID]),
        gw2=np.ascontiguousarray(gate_w[c.HID:]),
        out_w=np.asarray(inputs["out_w"], np.float32),
        gate_b=np.asarray(inputs["gate_b"], np.float32).reshape(c.HID, 1),
        aux_rows=aux,
        ln_params=lnp,
        # raw (for specialization flags)
        in_ln_g=inputs["in_ln_g"], in_ln_b=inputs["in_ln_b"],
        ln_g=np.asarray(inputs["ln_g"]), ln_b=np.asarray(inputs["ln_b"]),
        out_ln_g=inputs["out_ln_g"], out_ln_b=inputs["out_ln_b"],
        gcn_b=np.asarray(inputs["gcn_b"]), out_b=inputs["out_b"],
    )


def kernel_impl(inputs, cfg, profile=False):
    global LAST_EXEC_NS, LAST_RESULTS
    inputs = {k: np.asarray(v) for k, v in inputs.items()}
    per_core, meta = host_prep(inputs, cfg)
    wts = _weights_pack(inputs, cfg)

    key = (cfg.N, cfg.E, cfg.C,
           hash(inputs["edge_index"].tobytes()),
           hash(inputs["batch"].tobytes()),
           meta["s_scale"], meta["dt"], meta["rw"])
    if key not in _CACHE:
        nc = build_kernel(cfg, meta, wts)
        if not nc.is_finalized():
            nc.finalize()
        _CACHE.clear()
        _CACHE[key] = nc
    nc = _CACHE[key]

    in_maps = []
    for q in range(cfg.C):
        m = dict(per_core[q])
        m["in_w65"] = wts["in_w65"]
        for i in range(3):
            m[f"w{i}"] = wts["w"][i]
        m["gw1"] = wts["gw1"]
        m["gw2"] = wts["gw2"]
        m["out_w"] = wts["out_w"]
        m["gate_b"] = wts["gate_b"]
        m["aux_rows"] = wts["aux_rows"]
        m["ln_params"] = wts["ln_params"]
        m["idxA"] = m.pop("idxA")
        m["idxB"] = m.pop("idxB")
        m = {k: v for k, v in m.items()}
        in_maps.append(m)

    res = run_bass_kernel_spmd(nc, in_maps, core_ids=list(range(cfg.C)),
                               trace=profile)
    LAST_RESULTS = res
    LAST_EXEC_NS = res.exec_time_ns

    pooled = np.zeros((cfg.G, cfg.HID), np.float64)
    for q in range(cfg.C):
        pooled += np.asarray(res.results[q]["pool_out"], np.float64)
    cnt = np.maximum(meta["cnt_g"], 1.0)
    out = (pooled / cnt[:, None]).astype(np.float32)
    return out


def kernel(**inputs):
    cfg = Cfg(N=50000, E=800000, FEAT=64, HID=128, G=8, C=8)
    profile = bool(int(os.environ.get("GNN_PROFILE", "0")))
    return kernel_impl(inputs, cfg, profile=profile)

